# revision 1
# baseline (speedup 1.0000x reference)
"""Trainium2 Bass kernel for nn_MultiHeadLiftLayer (GNN edge-signal lift).

Computes, for each edge e with endpoints (src, tgt):
    out[e, k] = relu( x[src] . a_src[k]  +  x[tgt] . a_tgt[k] ),  k = 0..3

Strategy (edge-parallel across 8 NeuronCores):
  - Each core first computes the per-node projection table
    p[n, :] = [x[n] @ a_src.T | x[n] @ a_tgt.T]  (shape [NODES, 8], f32)
    with PE matmuls (node-major PSUM output, strided weight loads so the
    p-table store DMA is contiguous), and writes it to a DRAM scratch.
  - Edges are sharded 8 ways. Each core gathers p_src[src[e]] and
    p_tgt[tgt[e]] with indirect DMAs: the TRN2 SWDGE consumes one offset
    per destination partition-row, so each instruction gathers 128 rows
    (one edge per partition). Index transform 2*i / 2*i+1 against a
    (2N, 4) view of the table bakes the src/tgt column offset into the
    index. A DVE add + ACT relu fuse the two, and results are DMA'd out
    contiguously.
"""

import numpy as np

import concourse.bacc as bacc
import concourse.bass as bass
import concourse.mybir as mybir
import concourse.tile as tile
from concourse.bass import IndirectOffsetOnAxis
from concourse.bass_utils import run_bass_kernel_spmd

# ---- problem constants (hardcoded per contract) ----
N_NODES = 50000
N_EDGES = 800000
F_IN = 64
K = 4
CORES = 8

# phase 1 tiling: blocks of 2048 nodes = 128 partitions x 16 node-chunks
BLK_W = 16
BLK = 128 * BLK_W            # 2048
N_BLOCKS = 13
N_HALF = N_BLOCKS * BLK      # 26624 padded half
NODES_PAD = 2 * N_HALF       # 53248 >= 50000

# phase 2 tiling: per-core edge shard laid out [128, M]
E_C = N_EDGES // CORES       # 100000
M = (E_C + 127) // 128       # 782
E_PAD = 128 * M              # 100096
M_CHUNKS = [98] * 7 + [96]   # sum = 782

F32 = mybir.dt.float32
I32 = mybir.dt.int32

_PROGRAM_CACHE = {}


def _build_program():
    nc = bacc.Bacc("TRN2")

    x2 = nc.dram_tensor("x2", [128, N_HALF], F32, kind="ExternalInput")
    a_in = nc.dram_tensor("a_in", [128, 8], F32, kind="ExternalInput")
    src_in = nc.dram_tensor("src_idx", [128, M], I32, kind="ExternalInput")
    tgt_in = nc.dram_tensor("tgt_idx", [128, M], I32, kind="ExternalInput")
    out_d = nc.dram_tensor("out", [128, 4 * M], F32, kind="ExternalOutput")
    p_d = nc.dram_tensor("p_tab", [NODES_PAD, 8], F32)

    with tile.TileContext(nc) as tc:
        with (
            tc.tile_pool(name="const", bufs=1) as cpool,
            tc.tile_pool(name="xin", bufs=3) as xpool,
            tc.tile_pool(name="ps", bufs=4, space="PSUM") as ppool,
            tc.tile_pool(name="pstage", bufs=3) as spool,
            tc.tile_pool(name="gath", bufs=8) as gpool,
            tc.tile_pool(name="rel", bufs=6) as rpool,
        ):
            # PE LDWEIGHTS supports only a single sync-wait, so PE operands
            # are staged through DVE copies: every matmul dependency then
            # rides the single DVE semaphore lane.
            a_raw = cpool.tile([128, 8], F32)
            nc.sync.dma_start(out=a_raw[:], in_=a_in[:])
            a_sb = cpool.tile([128, 8], F32)
            nc.vector.tensor_copy(out=a_sb[:], in_=a_raw[:])
            src_sb = cpool.tile([128, M], I32)
            tgt_sb = cpool.tile([128, M], I32)
            nc.sync.dma_start(out=src_sb[:], in_=src_in[:])
            nc.sync.dma_start(out=tgt_sb[:], in_=tgt_in[:])

            # ---- phase 1: p[n, :] = x[n] @ A  (node-major) ----
            store_insts = []
            for b in range(N_BLOCKS):
                xtr = xpool.tile([128, BLK], F32, tag="xtr")
                nc.sync.dma_start(out=xtr[:], in_=x2[:, b * BLK:(b + 1) * BLK])
                xt = xpool.tile([128, BLK], F32, tag="xt")
                nc.vector.tensor_copy(out=xt[:], in_=xtr[:])
                for h in (0, 1):
                    pt = ppool.tile([128, BLK_W * 8], F32)
                    for w in range(BLK_W):
                        # node n = h*N_HALF + b*BLK + 16*j + w sits in
                        # lhsT column j -> PSUM partition j
                        lhsT = xt[64 * h:64 * h + 64,
                                  w:w + BLK - BLK_W + 1:BLK_W]
                        nc.tensor.matmul(
                            out=pt[:, 8 * w:8 * w + 8],
                            lhsT=lhsT,
                            rhs=a_sb[64 * h:64 * h + 64, :],
                            start=True,
                            stop=True,
                        )
                    st = spool.tile([128, BLK_W * 8], F32)
                    nc.vector.tensor_copy(out=st[:], in_=pt[:])
                    base = h * N_HALF + b * BLK
                    dst = p_d[base:base + BLK, :].rearrange(
                        "(j w) k -> j w k", j=128
                    )
                    ins = nc.sync.dma_start(
                        out=dst, in_=st[:].rearrange("p (w k) -> p w k", k=8)
                    )
                    store_insts.append(ins.ins)

            # ---- phase 2: gather + add + relu per edge tile ----
            # join all p-table stores into one Pool-engine nop so the
            # gathers don't each carry 8 DMA-lane sync waits
            p_join = nc.engines[mybir.EngineType.Pool].nop(
                nofuse=True, hint="p_join"
            )
            for s in store_insts:
                tile.add_dep_helper(p_join.ins, s, reason="join p stores")
            # view p as (2*NODES_PAD, 4): row 2n = p_src(n), 2n+1 = p_tgt(n)
            p_view = p_d[:, :].rearrange("n (two k) -> (n two) k", two=2)
            mo = 0
            for mc in M_CHUNKS:
                ga = gpool.tile([128, 4 * mc], F32)
                gb = gpool.tile([128, 4 * mc], F32)
                # HW indirect DMA consumes one offset per dest partition-row,
                # so gather 128 rows (one per partition) per instruction.
                for m in range(mc):
                    i1 = nc.gpsimd.indirect_dma_start(
                        out=ga[:, 4 * m:4 * m + 4],
                        out_offset=None,
                        in_=p_view,
                        in_offset=IndirectOffsetOnAxis(
                            ap=src_sb[:, mo + m:mo + m + 1], axis=0
                        ),
                    )
                    i2 = nc.gpsimd.indirect_dma_start(
                        out=gb[:, 4 * m:4 * m + 4],
                        out_offset=None,
                        in_=p_view,
                        in_offset=IndirectOffsetOnAxis(
                            ap=tgt_sb[:, mo + m:mo + m + 1], axis=0
                        ),
                    )
                    if m == 0:
                        tile.add_dep_helper(
                            i1.ins, p_join.ins, reason="gather after p"
                        )
                        tile.add_dep_helper(
                            i2.ins, p_join.ins, reason="gather after p"
                        )
                sm = rpool.tile([128, 4 * mc], F32)
                nc.vector.tensor_tensor(
                    out=sm[:], in0=ga[:], in1=gb[:], op=mybir.AluOpType.add
                )
                r = rpool.tile([128, 4 * mc], F32)
                nc.scalar.activation(
                    out=r[:], in_=sm[:],
                    func=mybir.ActivationFunctionType.Relu,
                )
                nc.sync.dma_start(out=out_d[:, 4 * mo:4 * (mo + mc)], in_=r[:])
                mo += mc

    nc.compile()
    return nc


def get_program():
    if "nc" not in _PROGRAM_CACHE:
        _PROGRAM_CACHE["nc"] = _build_program()
    return _PROGRAM_CACHE["nc"]


def make_in_maps(x, edge_index, att):
    """Marshal full inputs into per-core input maps."""
    x = np.asarray(x, dtype=np.float32)
    att = np.asarray(att, dtype=np.float32)
    ei = np.asarray(edge_index).astype(np.int64)

    # x2: [128, N_HALF]; rows 0-63 = features of nodes [0, N_HALF),
    # rows 64-127 = features of nodes [N_HALF, 2*N_HALF). zero-padded.
    xT = np.zeros((F_IN, NODES_PAD), dtype=np.float32)
    xT[:, :N_NODES] = x.T
    x2 = np.concatenate([xT[:, :N_HALF], xT[:, N_HALF:]], axis=0)
    x2 = np.ascontiguousarray(x2)

    # A: [64, 8] = [a_src.T | a_tgt.T], replicated on both partition halves
    A1 = np.empty((F_IN, 2 * K), dtype=np.float32)
    A1[:, :K] = att[:, :F_IN].T
    A1[:, K:] = att[:, F_IN:].T
    A = np.concatenate([A1, A1], axis=0)

    in_maps = []
    for c in range(CORES):
        s = ei[0, c * E_C:(c + 1) * E_C]
        t = ei[1, c * E_C:(c + 1) * E_C]
        # index transform for the (2N, 4) table view
        sp = np.zeros(E_PAD, dtype=np.int32)
        tp = np.zeros(E_PAD, dtype=np.int32)
        sp[:E_C] = 2 * s
        tp[:E_C] = 2 * t + 1
        in_maps.append({
            "x2": x2,
            "a_in": A,
            "src_idx": np.ascontiguousarray(sp.reshape(128, M)),
            "tgt_idx": np.ascontiguousarray(tp.reshape(128, M)),
        })
    return in_maps


def assemble_output(results):
    outs = []
    for c in range(CORES):
        o = np.asarray(results[c]["out"]).reshape(E_PAD, K)[:E_C]
        outs.append(o)
    return np.ascontiguousarray(np.concatenate(outs, axis=0))


def kernel(x, edge_index, att):
    nc = get_program()
    in_maps = make_in_maps(x, edge_index, att)
    res = run_bass_kernel_spmd(nc, in_maps, core_ids=list(range(CORES)))
    return assemble_output(res.results)



# revision 8
# speedup vs baseline: 3.6696x; 3.6696x over previous
"""Trainium2 Bass kernel for nn_MultiHeadLiftLayer (GNN edge-signal lift).

Computes, for each edge e with endpoints (src, tgt):
    out[e, k] = relu( x[src] . a_src[k]  +  x[tgt] . a_tgt[k] ),  k = 0..3

Strategy (edge-parallel across 8 NeuronCores):
  - Edges are sharded 8 ways (contiguous 100K slices).
  - Per core, each edge endpoint's x row (64 fp16 values padded to 128 =
    256B, the SWDGE dma_gather minimum element) is fetched with batched
    dma_gather instructions in TRANSPOSE mode: one instruction gathers
    4096 rows and lands them feature-major [128 feats, 4096 edges] in
    SBUF, ready to be the PE matmul moving operand.
  - The per-edge projection + add comes free on the PE: psum[4, e] is
    accumulated over two matmuls (a_src.T @ Xs then a_tgt.T @ Xt with
    start/stop accumulation), then ACT applies relu and the [4, e]
    K-major result is DMA'd out. The host transposes back to (E, 4).
  - dma_gather indices are int16 (max 32767) but N=50000, so x is staged
    as TWO half-tables of 26624 rows and edges are bucketed host-side by
    (src-half, tgt-half) into 4 buckets; each bucket does its src gather
    from table hs and tgt gather from table ht with half-local indices.
    Bucket slots are padded to a fixed capacity with index 0 (valid row,
    results dropped on host) so the program stays static. In the
    (pathological) case a bucket overflows its capacity, the same
    program is simply run again on the leftover edges.
"""

import numpy as np

import concourse.bacc as bacc
import concourse.mybir as mybir
import concourse.tile as tile
from concourse.bass_utils import run_bass_kernel_spmd

# ---- problem constants (hardcoded per contract) ----
N_NODES = 50000
N_EDGES = 800000
F_IN = 64
K = 4
CORES = 8

SPLIT = 25000                # node id threshold between the two halves
NH = 26624                   # rows per half-table (>= SPLIT)
E_C = N_EDGES // CORES       # 100000 edges per core
CAP = 26624                  # bucket capacity (multiple of 4096 + 2048)
CHUNK = 512                  # SWDGE ring limit for transpose dma_gather
NCHUNK = CAP // CHUNK        # 52 chunks per bucket-side
ICOLS = CAP // 16            # idx columns per bucket-side (wrapped layout)
MM = 512                     # psum sub-chunk (PSUM bank = 512 f32)

F32 = mybir.dt.float32
F16 = mybir.dt.float16
I16 = mybir.dt.int16

_PROGRAM_CACHE = {}


def _build_program():
    nc = bacc.Bacc("TRN2", num_swdge_queues=4)

    tb = [
        nc.dram_tensor(f"tb{h}", [NH, 128], F16, kind="ExternalInput")
        for h in (0, 1)
    ]
    a_in = nc.dram_tensor("a_in", [64, 8], F16, kind="ExternalInput")
    # 8 bucket-sides packed: [(b0,src),(b0,tgt),(b1,src),...] each ICOLS wide
    idx_in = nc.dram_tensor("idx_in", [128, 8 * ICOLS], I16,
                            kind="ExternalInput")
    out_d = nc.dram_tensor("out", [4, 4 * CAP], F32, kind="ExternalOutput")

    with tile.TileContext(nc) as tc:
        with (
            tc.tile_pool(name="const", bufs=1) as cpool,
            tc.tile_pool(name="gath", bufs=16) as gpool,
            tc.tile_pool(name="ps", bufs=8, space="PSUM") as ppool,
            tc.tile_pool(name="rel", bufs=3) as rpool,
        ):
            # stage PE weights through a DVE copy so matmul deps ride the
            # single-sync-wait LDWEIGHTS path cleanly
            a_raw = cpool.tile([64, 8], F16)
            nc.sync.dma_start(out=a_raw[:], in_=a_in[:])
            a_sb = cpool.tile([64, 8], F16)
            nc.vector.tensor_copy(out=a_sb[:], in_=a_raw[:])

            idx_sb = cpool.tile([128, 8 * ICOLS], I16)
            nc.sync.dma_start(out=idx_sb[:], in_=idx_in[:])

            qn = 0
            for b in range(4):
                hs, ht = b >> 1, b & 1
                # group 8 chunks (4096 edges) per output store
                for grp in range((NCHUNK + 7) // 8):
                    glen = min(8, NCHUNK - grp * 8)
                    r = rpool.tile([4, glen * CHUNK], F32)
                    for ci in range(glen):
                        off = (grp * 8 + ci) * CHUNK
                        xg = []
                        for side, h in ((0, hs), (1, ht)):
                            g = gpool.tile([128, CHUNK], F16,
                                           tag=f"g{side}")
                            c0 = (2 * b + side) * ICOLS + off // 16
                            nc.gpsimd.dma_gather(
                                out_ap=g[:].rearrange(
                                    "p (o m) -> p o m", o=1),
                                in_ap=tb[h][:, :],
                                idxs_ap=idx_sb[:, c0:c0 + CHUNK // 16],
                                num_idxs=CHUNK,
                                num_idxs_reg=CHUNK,
                                elem_size=128,
                                transpose=True,
                                queue_num=qn % 4,
                            )
                            qn += 1
                            xg.append(g)
                        ps = ppool.tile([4, CHUNK], F32)
                        nc.tensor.matmul(
                            out=ps[:],
                            lhsT=a_sb[:, 0:4],
                            rhs=xg[0][0:64, :],
                            start=True,
                            stop=False,
                        )
                        nc.tensor.matmul(
                            out=ps[:],
                            lhsT=a_sb[:, 4:8],
                            rhs=xg[1][0:64, :],
                            start=False,
                            stop=True,
                        )
                        nc.scalar.activation(
                            out=r[:, ci * CHUNK:(ci + 1) * CHUNK],
                            in_=ps[:],
                            func=mybir.ActivationFunctionType.Relu,
                        )
                    o0 = b * CAP + grp * 8 * CHUNK
                    nc.sync.dma_start(
                        out=out_d[:, o0:o0 + glen * CHUNK], in_=r[:],
                    )

    nc.compile()
    return nc


def get_program():
    if "nc" not in _PROGRAM_CACHE:
        _PROGRAM_CACHE["nc"] = _build_program()
    return _PROGRAM_CACHE["nc"]


def _wrap_idx(lst):
    """Wrap an index list (len CAP) for SWDGE: idx j -> [16g + j%16, j//16]
    replicated across the 8 GPSIMD cores (g = 0..7)."""
    w = lst.reshape(ICOLS, 16).T.astype(np.int16)   # [16, ICOLS]
    return np.tile(w, (8, 1))                       # [128, ICOLS]


def make_tables(x, att):
    x = np.asarray(x, dtype=np.float32)
    att = np.asarray(att, dtype=np.float32)
    xt = np.zeros((SPLIT + NH, 128), dtype=np.float16)
    xt[:N_NODES, :F_IN] = x.astype(np.float16)
    a = np.empty((F_IN, 8), dtype=np.float16)
    a[:, :K] = att[:, :F_IN].T.astype(np.float16)
    a[:, K:] = att[:, F_IN:].T.astype(np.float16)
    return xt[:NH], xt[SPLIT:SPLIT + NH], a


def prepare_passes(x, edge_index, att):
    """Host marshaling: bucket/pad per-core edges, build per-pass in_maps.

    Returns a list of (in_maps, slot_maps) passes; slot_maps[c] is a list of
    (bucket, edge_ids) giving which original edge each output slot holds.
    Normally a single pass; more only if a bucket overflows CAP.
    """
    tb0, tb1, a = make_tables(x, att)
    ei = np.asarray(edge_index).astype(np.int64)

    core_state = []
    for c in range(CORES):
        s = ei[0, c * E_C:(c + 1) * E_C].astype(np.int64)
        t = ei[1, c * E_C:(c + 1) * E_C].astype(np.int64)
        bid = (s >= SPLIT) * 2 + (t >= SPLIT)
        order = np.argsort(bid, kind="stable")
        counts = np.bincount(bid, minlength=4)
        core_state.append((s, t, order, counts))

    n_pass = max(
        1, int(np.ceil(max(cs[3].max() for cs in core_state) / CAP))
    )
    passes = []
    for p in range(n_pass):
        in_maps = []
        slot_maps = []
        for c in range(CORES):
            s, t, order, counts = core_state[c]
            idx_arr = np.zeros((128, 8 * ICOLS), dtype=np.int16)
            slots = []  # (bucket, edge_ids) for this pass
            cum = np.concatenate([[0], np.cumsum(counts)])
            for b in range(4):
                lo = cum[b] + p * CAP
                hi = min(cum[b] + counts[b], lo + CAP)
                eids = order[lo:hi] if lo < hi else np.empty(0, np.int64)
                sl = np.zeros(CAP, dtype=np.int64)
                tl = np.zeros(CAP, dtype=np.int64)
                sl[:len(eids)] = s[eids] - (b >> 1) * SPLIT
                tl[:len(eids)] = t[eids] - (b & 1) * SPLIT
                idx_arr[:, (2 * b) * ICOLS:(2 * b + 1) * ICOLS] = \
                    _wrap_idx(sl)
                idx_arr[:, (2 * b + 1) * ICOLS:(2 * b + 2) * ICOLS] = \
                    _wrap_idx(tl)
                slots.append((b, eids))
            in_maps.append({
                "tb0": tb0, "tb1": tb1, "a_in": a, "idx_in": idx_arr,
            })
            slot_maps.append(slots)
        passes.append((in_maps, slot_maps))
    return passes


TRACE = False           # test harness hook: set True to request NTFF trace
LAST_RESULTS = []       # test harness hook: BassSpmdResult of each pass


def kernel(x, edge_index, att):
    nc = get_program()
    out = np.empty((N_EDGES, K), dtype=np.float32)
    LAST_RESULTS.clear()
    for in_maps, slot_maps in prepare_passes(x, edge_index, att):
        res = run_bass_kernel_spmd(
            nc, in_maps, core_ids=list(range(CORES)), trace=TRACE
        )
        LAST_RESULTS.append(res)
        for c in range(CORES):
            o = np.asarray(res.results[c]["out"])  # [4, 4*CAP]
            for b, eids in slot_maps[c]:
                if len(eids):
                    out[c * E_C + eids] = o[:, b * CAP:b * CAP + len(eids)].T
    return out


# revision 13
# speedup vs baseline: 3.6808x; 1.0031x over previous
"""Trainium2 Bass kernel for nn_MultiHeadLiftLayer (GNN edge-signal lift).

Computes, for each edge e with endpoints (src, tgt):
    out[e, k] = relu( x[src] . a_src[k]  +  x[tgt] . a_tgt[k] ),  k = 0..3

Strategy (edge-parallel across 8 NeuronCores):
  - Edges are sharded 8 ways (contiguous 100K slices).
  - Per core, each edge endpoint's x row (64 fp16 values padded to 128 =
    256B, the SWDGE dma_gather minimum element) is fetched with batched
    dma_gather instructions in TRANSPOSE mode: one instruction gathers
    4096 rows and lands them feature-major [128 feats, 4096 edges] in
    SBUF, ready to be the PE matmul moving operand.
  - The per-edge projection + add comes free on the PE: psum[4, e] is
    accumulated over two matmuls (a_src.T @ Xs then a_tgt.T @ Xt with
    start/stop accumulation), then ACT applies relu and the [4, e]
    K-major result is DMA'd out. The host transposes back to (E, 4).
  - dma_gather indices are int16 (max 32767) but N=50000, so x is staged
    as TWO half-tables of 26624 rows and edges are bucketed host-side by
    (src-half, tgt-half) into 4 buckets; each bucket does its src gather
    from table hs and tgt gather from table ht with half-local indices.
    Bucket slots are padded to a fixed capacity with index 0 (valid row,
    results dropped on host) so the program stays static. In the
    (pathological) case a bucket overflows its capacity, the same
    program is simply run again on the leftover edges.
"""

import numpy as np

import concourse.bacc as bacc
import concourse.mybir as mybir
import concourse.tile as tile
from concourse.bass_utils import run_bass_kernel_spmd

# ---- problem constants (hardcoded per contract) ----
N_NODES = 50000
N_EDGES = 800000
F_IN = 64
K = 4
CORES = 8

SPLIT = 25000                # node id threshold between the two halves
NH = 26624                   # rows per half-table (>= SPLIT)
E_C = N_EDGES // CORES       # 100000 edges per core
CHUNK = 512                  # num_idxs per transpose dma_gather; two of
                             # these per SWDGE queue ring is the safe limit
NCHUNK = 52                  # chunks per bucket-side
CAP = CHUNK * NCHUNK         # 26624 bucket capacity
ICOLS = CAP // 16            # idx columns per bucket-side (wrapped layout)
MM = 512                     # psum sub-chunk (PSUM bank = 512 f32)

F32 = mybir.dt.float32
F16 = mybir.dt.float16
I16 = mybir.dt.int16

_PROGRAM_CACHE = {}


def _build_program():
    nc = bacc.Bacc("TRN2", num_swdge_queues=4)

    tb = [
        nc.dram_tensor(f"tb{h}", [NH, 128], F16, kind="ExternalInput")
        for h in (0, 1)
    ]
    a_in = nc.dram_tensor("a_in", [64, 8], F16, kind="ExternalInput")
    # 8 bucket-sides packed: [(b0,src),(b0,tgt),(b1,src),...] each ICOLS wide
    idx_in = nc.dram_tensor("idx_in", [128, 8 * ICOLS], I16,
                            kind="ExternalInput")
    out_d = nc.dram_tensor("out", [4, 4 * CAP], F32, kind="ExternalOutput")

    with tile.TileContext(nc) as tc:
        with (
            tc.tile_pool(name="const", bufs=1) as cpool,
            tc.tile_pool(name="gath", bufs=16) as gpool,
            tc.tile_pool(name="ps", bufs=8, space="PSUM") as ppool,
            tc.tile_pool(name="rel", bufs=3) as rpool,
        ):
            # stage PE weights through a DVE copy so matmul deps ride the
            # single-sync-wait LDWEIGHTS path cleanly
            a_raw = cpool.tile([64, 8], F16)
            nc.sync.dma_start(out=a_raw[:], in_=a_in[:])
            a_sb = cpool.tile([64, 8], F16)
            nc.vector.tensor_copy(out=a_sb[:], in_=a_raw[:])

            idx_sb = cpool.tile([128, 8 * ICOLS], I16)
            nc.sync.dma_start(out=idx_sb[:], in_=idx_in[:])

            qn = 0
            for b in range(4):
                hs, ht = b >> 1, b & 1
                for ci in range(NCHUNK):
                    off = ci * CHUNK
                    xg = []
                    for side, h in ((0, hs), (1, ht)):
                        g = gpool.tile([128, CHUNK], F16, tag=f"g{side}")
                        c0 = (2 * b + side) * ICOLS + off // 16
                        nc.gpsimd.dma_gather(
                            out_ap=g[:].rearrange("p (o m) -> p o m", o=1),
                            in_ap=tb[h][:, :],
                            idxs_ap=idx_sb[:, c0:c0 + CHUNK // 16],
                            num_idxs=CHUNK,
                            num_idxs_reg=CHUNK,
                            elem_size=128,
                            transpose=True,
                            queue_num=qn % 4,
                        )
                        qn += 1
                        xg.append(g)
                    r = rpool.tile([4, CHUNK], F32)
                    ps = ppool.tile([4, MM], F32)
                    nc.tensor.matmul(
                        out=ps[:],
                        lhsT=a_sb[:, 0:4],
                        rhs=xg[0][0:64, :],
                        start=True,
                        stop=False,
                    )
                    nc.tensor.matmul(
                        out=ps[:],
                        lhsT=a_sb[:, 4:8],
                        rhs=xg[1][0:64, :],
                        start=False,
                        stop=True,
                    )
                    nc.scalar.activation(
                        out=r[:], in_=ps[:],
                        func=mybir.ActivationFunctionType.Relu,
                    )
                    o0 = b * CAP + off
                    nc.sync.dma_start(
                        out=out_d[:, o0:o0 + CHUNK], in_=r[:],
                    )

    nc.compile()
    return nc


def get_program():
    if "nc" not in _PROGRAM_CACHE:
        _PROGRAM_CACHE["nc"] = _build_program()
    return _PROGRAM_CACHE["nc"]


def _wrap_idx(lst):
    """Wrap an index list (len CAP) for SWDGE: idx j -> [16g + j%16, j//16]
    replicated across the 8 GPSIMD cores (g = 0..7)."""
    w = lst.reshape(ICOLS, 16).T.astype(np.int16)   # [16, ICOLS]
    return np.tile(w, (8, 1))                       # [128, ICOLS]


def make_tables(x, att):
    x = np.asarray(x, dtype=np.float32)
    att = np.asarray(att, dtype=np.float32)
    xt = np.zeros((SPLIT + NH, 128), dtype=np.float16)
    xt[:N_NODES, :F_IN] = x.astype(np.float16)
    a = np.empty((F_IN, 8), dtype=np.float16)
    a[:, :K] = att[:, :F_IN].T.astype(np.float16)
    a[:, K:] = att[:, F_IN:].T.astype(np.float16)
    return xt[:NH], xt[SPLIT:SPLIT + NH], a


def prepare_passes(x, edge_index, att):
    """Host marshaling: bucket/pad per-core edges, build per-pass in_maps.

    Returns a list of (in_maps, slot_maps) passes; slot_maps[c] is a list of
    (bucket, edge_ids) giving which original edge each output slot holds.
    Normally a single pass; more only if a bucket overflows CAP.
    """
    tb0, tb1, a = make_tables(x, att)
    ei = np.asarray(edge_index).astype(np.int64)

    core_state = []
    for c in range(CORES):
        s = ei[0, c * E_C:(c + 1) * E_C].astype(np.int64)
        t = ei[1, c * E_C:(c + 1) * E_C].astype(np.int64)
        bid = (s >= SPLIT) * 2 + (t >= SPLIT)
        order = np.argsort(bid, kind="stable")
        counts = np.bincount(bid, minlength=4)
        core_state.append((s, t, order, counts))

    n_pass = max(
        1, int(np.ceil(max(cs[3].max() for cs in core_state) / CAP))
    )
    passes = []
    for p in range(n_pass):
        in_maps = []
        slot_maps = []
        for c in range(CORES):
            s, t, order, counts = core_state[c]
            idx_arr = np.zeros((128, 8 * ICOLS), dtype=np.int16)
            slots = []  # (bucket, edge_ids) for this pass
            cum = np.concatenate([[0], np.cumsum(counts)])
            for b in range(4):
                lo = cum[b] + p * CAP
                hi = min(cum[b] + counts[b], lo + CAP)
                eids = order[lo:hi] if lo < hi else np.empty(0, np.int64)
                sl = np.zeros(CAP, dtype=np.int64)
                tl = np.zeros(CAP, dtype=np.int64)
                sl[:len(eids)] = s[eids] - (b >> 1) * SPLIT
                tl[:len(eids)] = t[eids] - (b & 1) * SPLIT
                idx_arr[:, (2 * b) * ICOLS:(2 * b + 1) * ICOLS] = \
                    _wrap_idx(sl)
                idx_arr[:, (2 * b + 1) * ICOLS:(2 * b + 2) * ICOLS] = \
                    _wrap_idx(tl)
                slots.append((b, eids))
            in_maps.append({
                "tb0": tb0, "tb1": tb1, "a_in": a, "idx_in": idx_arr,
            })
            slot_maps.append(slots)
        passes.append((in_maps, slot_maps))
    return passes


TRACE = False           # test harness hook: set True to request NTFF trace
LAST_RESULTS = []       # test harness hook: BassSpmdResult of each pass


def kernel(x, edge_index, att):
    nc = get_program()
    out = np.empty((N_EDGES, K), dtype=np.float32)
    LAST_RESULTS.clear()
    for in_maps, slot_maps in prepare_passes(x, edge_index, att):
        res = run_bass_kernel_spmd(
            nc, in_maps, core_ids=list(range(CORES)), trace=TRACE
        )
        LAST_RESULTS.append(res)
        for c in range(CORES):
            o = np.asarray(res.results[c]["out"])  # [4, 4*CAP]
            for b, eids in slot_maps[c]:
                if len(eids):
                    out[c * E_C + eids] = o[:, b * CAP:b * CAP + len(eids)].T
    return out
